# revision 1
# baseline (speedup 1.0000x reference)
"""MoE grouped-GEMM expert FFN (SwiGLU) on 8 Trainium2 NeuronCores.

Expert-parallel sharding: tokens arrive pre-grouped by expert with uniform
group size g = T/E = 1024, so core c owns experts [4c, 4c+4) and token rows
[c*4096, (c+1)*4096). No cross-core communication is needed: each core
computes its own 4 experts' FFN on its own token block.

Per-core math, per expert e:
    gu^T = w13_e^T-chunks @ x_e^T        # PE: contract H on partitions
    h^T  = silu(gate^T) * up^T           # ACT (Silu) + DVE (mul), bf16 out
    out  = h @ w2_e                      # PE: contract I on partitions

The host pre-transposes x (so H lands on SBUF partitions) and pre-tiles the
weights into [128, free] k-tiles, giving every DMA >=1KB contiguous
per-partition lines. All matmuls are 128x128 stationary x [128,512] moving,
bf16 in / fp32 PSUM accumulate.
"""

import sys

if "/opt/trn_rl_repo" not in sys.path:
    sys.path.insert(0, "/opt/trn_rl_repo")

import ml_dtypes
import numpy as np

import concourse.bacc as bacc
import concourse.bass as bass
import concourse.mybir as mybir
from concourse import tile
from concourse.bass_utils import run_bass_kernel_spmd

BF16 = mybir.dt.bfloat16
F32 = mybir.dt.float32
NPBF16 = ml_dtypes.bfloat16

N_CORES = 8
E = 32
H = 2048
I = 1024
T = 32768
EPC = E // N_CORES          # experts per core = 4
G = T // E                  # tokens per expert = 1024
ROWS = EPC * G              # token rows per core = 4096
KH = H // 128               # 16 contraction tiles for GEMM1
KI = I // 128               # 8 contraction tiles for GEMM2


def build_nc():
    nc = bacc.Bacc()
    xt_d = nc.declare_dram_parameter("xt", [KH, 128, ROWS], BF16, isOutput=False)
    w13_d = nc.declare_dram_parameter("w13", [EPC, KH, 128, 2 * I], BF16, isOutput=False)
    w2_d = nc.declare_dram_parameter("w2", [EPC, KI, 128, H], BF16, isOutput=False)
    out_d = nc.declare_dram_parameter("out", [ROWS, H], F32, isOutput=True)

    with tile.TileContext(nc) as tc:
        with (
            tc.tile_pool(name="xt", bufs=1) as xt_pool,
            tc.tile_pool(name="w13", bufs=1) as w13_pool,
            tc.tile_pool(name="w2", bufs=1) as w2_pool,
            tc.tile_pool(name="h", bufs=2) as h_pool,
            tc.tile_pool(name="tmp", bufs=3) as tmp_pool,
            tc.tile_pool(name="ost", bufs=4) as ost_pool,
            tc.tile_pool(name="ps", bufs=2, space="PSUM") as ps_pool,
        ):
            for e in range(EPC):
                xt_sb = []
                for k in range(KH):
                    t = xt_pool.tile([128, G], BF16, tag=f"xt{k}", bufs=1, name=f"xt{k}_{e}")
                    nc.sync.dma_start(t[:], xt_d[k][:, e * G:(e + 1) * G])
                    xt_sb.append(t)
                w13_sb = []
                for k in range(KH):
                    t = w13_pool.tile([128, 2 * I], BF16, tag=f"w13_{k}", bufs=1, name=f"w13_{k}_{e}")
                    nc.sync.dma_start(t[:], w13_d[e, k][:])
                    w13_sb.append(t)
                w2_sb = []
                for k in range(KI):
                    t = w2_pool.tile([128, H], BF16, tag=f"w2_{k}", bufs=1, name=f"w2_{k}_{e}")
                    nc.sync.dma_start(t[:], w2_d[e, k][:])
                    w2_sb.append(t)

                # Phase 1: gu^T tiles -> SwiGLU -> h^T resident in SBUF (bf16).
                h_sb = [h_pool.tile([128, G], BF16, tag=f"h{m}", bufs=2, name=f"h{m}_{e}") for m in range(KI)]
                for m in range(KI):
                    # One PSUM bank per (gate/up, n) group; the k-loop
                    # interleaves all four so each stationary weight tile
                    # feeds two consecutive matmuls (LDW reuse) and the PE
                    # pipelines fills across banks.
                    pg = [ps_pool.tile([128, 512], F32, tag=f"pg{n}", bufs=1, name=f"pg{n}_{e}_{m}")
                          for n in range(2)]
                    pu = [ps_pool.tile([128, 512], F32, tag=f"pu{n}", bufs=1, name=f"pu{n}_{e}_{m}")
                          for n in range(2)]
                    for k in range(KH):
                        wg = w13_sb[k][:, m * 128:(m + 1) * 128]
                        wu = w13_sb[k][:, I + m * 128:I + (m + 1) * 128]
                        for n in range(2):
                            nc.tensor.matmul(
                                pg[n][:], wg, xt_sb[k][:, n * 512:(n + 1) * 512],
                                start=(k == 0), stop=(k == KH - 1),
                            )
                        for n in range(2):
                            nc.tensor.matmul(
                                pu[n][:], wu, xt_sb[k][:, n * 512:(n + 1) * 512],
                                start=(k == 0), stop=(k == KH - 1),
                            )
                    for n in range(2):
                        ncol = slice(n * 512, (n + 1) * 512)
                        tmp = tmp_pool.tile([128, 512], F32, tag="tmp", bufs=3, name=f"tmp_{e}_{m}_{n}")
                        pu_sb = tmp_pool.tile([128, 512], F32, tag="pusb", bufs=3, name=f"pusb_{e}_{m}_{n}")
                        nc.scalar.activation(
                            tmp[:], pg[n][:], mybir.ActivationFunctionType.Silu
                        )
                        # Both epilogue producers run on ACT so the DVE mul
                        # carries ONE merged ACT wait (the TT instruction
                        # encoding only fits a single sync-wait).
                        nc.scalar.copy(pu_sb[:], pu[n][:])
                        nc.vector.tensor_mul(h_sb[m][:, ncol], tmp[:], pu_sb[:])

                # Phase 2: out_e = h @ w2_e, streamed straight to DRAM.
                for mt in range(KI):
                    rows = slice(e * G + mt * 128, e * G + (mt + 1) * 128)
                    po = [ps_pool.tile([128, 512], F32, tag=f"po{n}", bufs=1, name=f"po{n}_{e}_{mt}")
                          for n in range(4)]
                    for k in range(KI):
                        hk = h_sb[k][:, mt * 128:(mt + 1) * 128]
                        for n in range(4):
                            nc.tensor.matmul(
                                po[n][:], hk, w2_sb[k][:, n * 512:(n + 1) * 512],
                                start=(k == 0), stop=(k == KI - 1),
                            )
                    for n in range(4):
                        ncol = slice(n * 512, (n + 1) * 512)
                        ot = ost_pool.tile([128, 512], F32, tag="ot", bufs=4, name=f"ot_{e}_{mt}_{n}")
                        nc.vector.tensor_copy(ot[:], po[n][:])
                        nc.sync.dma_start(out_d[rows, ncol], ot[:])
    nc.compile()
    return nc


def _in_map_for_core(x, w13, w2, c):
    xs = x[c * ROWS:(c + 1) * ROWS]                      # [4096, 2048] f32
    xt = xs.T.astype(NPBF16, order="C").reshape(KH, 128, ROWS)
    w13c = np.ascontiguousarray(w13[c * EPC:(c + 1) * EPC]).astype(NPBF16)
    w2c = np.ascontiguousarray(w2[c * EPC:(c + 1) * EPC]).astype(NPBF16)
    return {
        "xt": xt,
        "w13": w13c.reshape(EPC, KH, 128, 2 * I),
        "w2": w2c.reshape(EPC, KI, 128, H),
    }


def kernel(x, w13, w2, tokens_per_expert, decoding, _trace=False):
    x = np.asarray(x, dtype=np.float32)
    w13 = np.asarray(w13, dtype=np.float32)
    w2 = np.asarray(w2, dtype=np.float32)

    in_maps = [_in_map_for_core(x, w13, w2, c) for c in range(N_CORES)]
    nc = build_nc()
    res = run_bass_kernel_spmd(nc, in_maps, list(range(N_CORES)), trace=_trace)
    out = np.concatenate([res.results[c]["out"] for c in range(N_CORES)], axis=0)
    if _trace:
        return out, res
    return out



# revision 3
# speedup vs baseline: 1.8433x; 1.8433x over previous
"""MoE grouped-GEMM expert FFN (SwiGLU) on 8 Trainium2 NeuronCores.

Expert-parallel sharding: tokens arrive pre-grouped by expert with uniform
group size g = T/E = 1024, so core c owns experts [4c, 4c+4) and token rows
[c*4096, (c+1)*4096). No cross-core communication is needed.

Per-core schedule (per expert e):
    phase1: gu^T[m] = sum_k w13[e,k,m]^T @ xt[k]      (PE, 4 PSUM banks/m)
            h^T[m]  = silu(gate^T) * up^T             (ACT silu + DVE mul)
    phase2: out[mt] = sum_k h^T[k,mt]^T @ w2[e,k]     (PE, 4 PSUM banks/mt)
            bf16 copy to SBUF (DVE) -> DMA store      (ACT HWDGE ring)

Perf notes vs v1:
  - One rotating PSUM tag (8 banks, 4 banks/group -> 2-deep pipelining) so
    the PE never waits on the SwiGLU / output-copy epilogues.
  - Weights & activations stream with deep prefetch: per-expert batched
    DMAs issued in need-order on the SP ring; output stores ride the ACT
    ring so they can't head-of-line-block the prefetches.
  - DVE multiplies silu(gate) (SBUF) by up (PSUM directly) - no ACT copy.
  - Output is written bf16 (host upcasts) halving store traffic.
"""

import sys

if "/opt/trn_rl_repo" not in sys.path:
    sys.path.insert(0, "/opt/trn_rl_repo")

import ml_dtypes
import numpy as np

import concourse.bacc as bacc
import concourse.mybir as mybir
from concourse import tile
from concourse.bass_utils import run_bass_kernel_spmd

BF16 = mybir.dt.bfloat16
F32 = mybir.dt.float32
NPBF16 = ml_dtypes.bfloat16

N_CORES = 8
E = 32
H = 2048
I = 1024
T = 32768
EPC = E // N_CORES          # experts per core = 4
G = T // E                  # tokens per expert = 1024
ROWS = EPC * G              # token rows per core = 4096
KH = H // 128               # 16 contraction tiles for GEMM1
KI = I // 128               # 8 contraction tiles for GEMM2 / m-tiles


def build_nc():
    nc = bacc.Bacc()
    # xt:  x^T per expert, [e][k][128 h][1024 tok]
    xt_d = nc.declare_dram_parameter("xt", [EPC, KH, 128, G], BF16, isOutput=False)
    # w13: [e][m][128 j][k][s: gate|up][128 col] -> per (e,m) one contiguous 1MB
    w13_d = nc.declare_dram_parameter("w13", [EPC, KI, 128, KH, 2, 128], BF16, isOutput=False)
    # w2:  [e][k][128 i][2048 h]
    w2_d = nc.declare_dram_parameter("w2", [EPC, KI, 128, H], BF16, isOutput=False)
    out_d = nc.declare_dram_parameter("out", [ROWS, H], BF16, isOutput=True)

    with tile.TileContext(nc) as tc:
        with (
            tc.tile_pool(name="xt", bufs=2) as xt_pool,
            tc.tile_pool(name="w13", bufs=3) as w13_pool,
            tc.tile_pool(name="w2", bufs=10) as w2_pool,
            tc.tile_pool(name="h", bufs=2) as h_pool,
            tc.tile_pool(name="tmp", bufs=4) as tmp_pool,
            tc.tile_pool(name="ot", bufs=3) as ot_pool,
            tc.tile_pool(name="ps", bufs=8, space="PSUM") as ps_pool,
        ):
            for e in range(EPC):
                # ---- loads for expert e, in need-order on the SP ring ----
                xt_sb = xt_pool.tile([128, KH, G], BF16, tag="xt", name=f"xt_{e}")
                nc.sync.dma_start(xt_sb[:], xt_d[e].transpose([1, 0, 2]))

                w13_sb = []
                for m in range(KI):
                    t = w13_pool.tile([128, KH, 2, 128], BF16, tag="w13", name=f"w13_{e}_{m}")
                    nc.sync.dma_start(t[:], w13_d[e, m])
                    w13_sb.append(t)

                w2_sb = []
                for k in range(KI):
                    t = w2_pool.tile([128, H], BF16, tag="w2", name=f"w2_{e}_{k}")
                    nc.sync.dma_start(t[:], w2_d[e, k])
                    w2_sb.append(t)

                # ---- phase 1: h^T = silu(w1^T x^T) * (w3^T x^T), bf16 ----
                h_sb = h_pool.tile([128, KI, G], BF16, tag="h", name=f"h_{e}")
                for m in range(KI):
                    pg = [ps_pool.tile([128, 512], F32, tag="ps", name=f"pg{n}_{e}_{m}")
                          for n in range(2)]
                    pu = [ps_pool.tile([128, 512], F32, tag="ps", name=f"pu{n}_{e}_{m}")
                          for n in range(2)]
                    for k in range(KH):
                        wg = w13_sb[m][:, k, 0, :]
                        wu = w13_sb[m][:, k, 1, :]
                        for n in range(2):
                            nc.tensor.matmul(
                                pg[n][:], wg, xt_sb[:, k, n * 512:(n + 1) * 512],
                                start=(k == 0), stop=(k == KH - 1),
                            )
                        for n in range(2):
                            nc.tensor.matmul(
                                pu[n][:], wu, xt_sb[:, k, n * 512:(n + 1) * 512],
                                start=(k == 0), stop=(k == KH - 1),
                            )
                    for n in range(2):
                        tmp = tmp_pool.tile([128, 512], F32, tag="tmp", name=f"tmp_{e}_{m}_{n}")
                        nc.scalar.activation(
                            tmp[:], pg[n][:], mybir.ActivationFunctionType.Silu
                        )
                        nc.vector.tensor_mul(
                            h_sb[:, m, n * 512:(n + 1) * 512], tmp[:], pu[n][:]
                        )

                # ---- phase 2: out = h @ w2, bf16 stores on the ACT ring ----
                for mt in range(KI):
                    po = [ps_pool.tile([128, 512], F32, tag="ps", name=f"po{n}_{e}_{mt}")
                          for n in range(4)]
                    for k in range(KI):
                        hk = h_sb[:, k, mt * 128:(mt + 1) * 128]
                        for n in range(4):
                            nc.tensor.matmul(
                                po[n][:], hk, w2_sb[k][:, n * 512:(n + 1) * 512],
                                start=(k == 0), stop=(k == KI - 1),
                            )
                    ot = ot_pool.tile([128, H], BF16, tag="ot", name=f"ot_{e}_{mt}")
                    for n in range(4):
                        nc.vector.tensor_copy(ot[:, n * 512:(n + 1) * 512], po[n][:])
                    rows = slice(e * G + mt * 128, e * G + (mt + 1) * 128)
                    nc.scalar.dma_start(out_d[rows, :], ot[:])
    nc.compile()
    return nc


def _prep_inputs(x, w13, w2):
    """Host-side relayout for all cores at once (f32 -> bf16)."""
    # xt: [E, H, G] = per-expert x^T, then [E, KH, 128, G]
    xt = np.ascontiguousarray(
        x.reshape(E, G, H).transpose(0, 2, 1)
    ).astype(NPBF16).reshape(E, KH, 128, G)
    # w13: [E,H,2I] -> [E, k, p, s, m, j] -> [E, m, p, k, s, j]
    a13 = np.ascontiguousarray(
        w13.reshape(E, KH, 128, 2, KI, 128).transpose(0, 4, 2, 1, 3, 5)
    ).astype(NPBF16)
    a2 = w2.reshape(E, KI, 128, H).astype(NPBF16)
    return xt, a13, a2


def _in_map_for_core(xt, a13, a2, c):
    s = slice(c * EPC, (c + 1) * EPC)
    return {"xt": xt[s], "w13": a13[s], "w2": a2[s]}


_NC_CACHE = []


def kernel(x, w13, w2, tokens_per_expert, decoding, _trace=False):
    x = np.asarray(x, dtype=np.float32)
    w13 = np.asarray(w13, dtype=np.float32)
    w2 = np.asarray(w2, dtype=np.float32)

    xt, a13, a2 = _prep_inputs(x, w13, w2)
    in_maps = [_in_map_for_core(xt, a13, a2, c) for c in range(N_CORES)]
    if not _NC_CACHE:
        _NC_CACHE.append(build_nc())
    nc = _NC_CACHE[0]
    res = run_bass_kernel_spmd(nc, in_maps, list(range(N_CORES)), trace=_trace)
    out = np.concatenate(
        [np.asarray(res.results[c]["out"]).astype(np.float32) for c in range(N_CORES)],
        axis=0,
    )
    if _trace:
        return out, res
    return out
